# revision 11
# baseline (speedup 1.0000x reference)
"""DendriteLayer Trainium2 kernel.

Math (reference): out0 = x @ (w_in*w_in_mask).T + b_in; a = out0.reshape(B, dpc, out_dim);
winner = argmax_d(a * boost); out1 = a * one_hot(winner); y = out1f @ (w_out*dend_mask).T + b_out.

Sharding: 8 cores, core c owns global units u in [c*256, (c+1)*256) (all dpc=8 dendrites)
and output columns v with (v % 256) in [c*32, (c+1)*32). Both k-winners and the
block-diagonal output stage are then fully local to a core (no collectives).

Per-core j' layout is u'-major interleaved: j' = u'*8 + d, so the 8 dendrites of a
unit are consecutive, and each 512-wide chunk of j' is self-contained for both the
k-winners (max over d) and the output segment-sums.

The matmul runs as a SINGLE f32r (11-mantissa-bit) term: host pre-folds the sparsity
mask AND the k-winners boost into the weights (WB = rne11(w_in*mask*boost)), and
pre-rounds X to f32r. The PE then computes G_b = Xr @ WB directly boosted, so the
argmax needs no separate boost multiply, and the winner values come from
z = G_b * (w_out_elem / boost) with the boost divided out host-side. This cuts PE
work 3x vs an fp32-accurate hi/lo split; the f32r rounding perturbs the argmax for
~1.2e-4 of units, giving rel_err ~1.1e-2 (CPU-simulated, gate is 2e-2).

All weights stay resident in SBUF (16 f32r kt-strips, 128KB/partition), so X is
streamed exactly once (32MB) and total HBM traffic is ~53MB/core vs ~160MB for a
chunk-looped X. Stage-2 (max/is_ge/mul/segment-sum) reads G straight from PSUM.
"""

import numpy as np

B, IN_DIM, OUT_DIM, DPC = 4096, 2048, 2048, 8
ND = OUT_DIM * DPC
NCORES = 8
UPC = OUT_DIM // NCORES          # units per core = 256
JPC = UPC * DPC                  # j' per core = 2048
CHUNK = 512                      # j' chunk width (64 units x 8 dendrites)
NCHUNK = JPC // CHUNK            # 4
BT = 128                         # batch tile
NBT = B // BT                    # 32
KT = 128                         # k tile
NKT = IN_DIM // KT               # 16
YW = CHUNK // DPC                # y columns per chunk = 64
BOOST_STRENGTH = 2.0

_prog_cache = {}
LAST_RESULTS = None


def _round_f32r(a):
    """Round fp32 -> f32r (11 explicit mantissa bits), RNE. Values stay exactly
    representable so the PE's own f32r read rounding is a no-op."""
    u = np.ascontiguousarray(a, dtype=np.float32).view(np.uint32).astype(np.uint64)
    u = u + np.uint64(0x7FF) + ((u >> np.uint64(12)) & np.uint64(1))
    u = u & np.uint64(0xFFFFF000)
    return u.astype(np.uint32).view(np.float32)


def _build(has_bin, has_bout):
    import concourse.mybir as mybir
    import concourse.tile as tile
    from concourse import bacc

    f32 = mybir.dt.float32
    f32r = mybir.dt.float32r

    nc = bacc.Bacc("TRN2", target_bir_lowering=False, debug=False)
    bf16 = mybir.dt.bfloat16
    XT_d = nc.dram_tensor("XT", [IN_DIM, B], f32r, kind="ExternalInput").ap()
    WH_d = nc.dram_tensor("WH", [IN_DIM, JPC], bf16, kind="ExternalInput").ap()
    WL_d = nc.dram_tensor("WL", [IN_DIM, JPC], bf16, kind="ExternalInput").ap()
    We_d = nc.dram_tensor("We", [128, JPC], f32, kind="ExternalInput").ap()
    if has_bin:
        binb_d = nc.dram_tensor("binb", [128, JPC], f32, kind="ExternalInput").ap()
    if has_bout:
        bout_d = nc.dram_tensor("bout", [128, NCHUNK * YW], f32, kind="ExternalInput").ap()
    Y_d = nc.dram_tensor("Y", [B, NCHUNK, YW], f32, kind="ExternalOutput").ap()

    with tile.TileContext(nc) as tc:
        with tc.tile_pool(name="wres", bufs=1) as wres, \
             tc.tile_pool(name="whl", bufs=2) as whl, \
             tc.tile_pool(name="tbl", bufs=1) as tbl, \
             tc.tile_pool(name="xio", bufs=3) as xio, \
             tc.tile_pool(name="st2", bufs=2) as st2, \
             tc.tile_pool(name="ypool", bufs=3) as ypool, \
             tc.tile_pool(name="psum", bufs=8, space="PSUM") as psum:

            dma_engs = [nc.sync, nc.scalar, nc.gpsimd]

            def emit_x(i):
                xf = xio.tile([128, NKT * BT], f32r, name=f"xf_{i}", tag="xf")
                src = XT_d[:, i * BT:(i + 1) * BT].rearrange("(kt p) b -> p kt b", p=128)
                dst = xf[:].rearrange("p (kt b) -> p kt b", b=BT)
                h = NKT // 2
                nc.sync.dma_start(dst[:, :h, :], src[:, :h, :])
                nc.scalar.dma_start(dst[:, h:, :], src[:, h:, :])
                return xf

            # X for the first two (fused) b-tiles lands before the W strips queue
            x0 = emit_x(0)
            x1 = emit_x(1)

            # ---- resident masked+boosted f32r weights, 16 kt strips ----
            # Shipped as exact bf16 hi+lo halves (8MB instead of 16MB) and
            # reconstructed on the idle DVE during startup.
            wt = []
            for kt in range(NKT):
                wh = whl.tile([128, JPC], bf16, name=f"wh{kt}", tag="wh")
                wl = whl.tile([128, JPC], bf16, name=f"wl{kt}", tag="wl")
                dma_engs[kt % 3].dma_start(wh[:], WH_d[kt * KT:(kt + 1) * KT, :])
                dma_engs[(kt + 1) % 3].dma_start(wl[:], WL_d[kt * KT:(kt + 1) * KT, :])
                w_ = wres.tile([128, JPC], f32r, name=f"w{kt}", tag=f"w{kt}")
                nc.vector.tensor_add(w_[:], wh[:], wl[:])
                wt.append(w_)

            # ---- one-time tables (after strips on the gpsimd queue) ----
            we = tbl.tile([128, JPC], f32, name="we")
            nc.gpsimd.dma_start(we[:], We_d[:])
            if has_bin:
                binb = tbl.tile([128, JPC], f32, name="binb")
                nc.gpsimd.dma_start(binb[:], binb_d[:])
            if has_bout:
                bout = tbl.tile([128, NCHUNK * YW], f32, name="bout")
                nc.gpsimd.dma_start(bout[:], bout_d[:])

            x2 = emit_x(2)

            def emit_mm(i, xf, g):
                for kt in range(NKT):
                    lhsT = xf[:, kt * BT:(kt + 1) * BT]
                    for w in range(NCHUNK):
                        nc.tensor.matmul(g[w][:], lhsT, wt[kt][:, w * CHUNK:(w + 1) * CHUNK],
                                         start=(kt == 0), stop=(kt == NKT - 1))

            def emit_stage2(i, g, emul_eng):
                y = ypool.tile([128, NCHUNK * YW], f32, name=f"y_{i}", tag="y")
                for w in range(NCHUNK):
                    if has_bin:
                        gs = st2.tile([128, CHUNK], f32, name=f"gs_{i}_{w}", tag="gs")
                        nc.vector.tensor_add(gs[:], g[w][:], binb[:, w * CHUNK:(w + 1) * CHUNK])
                        gin = gs
                    else:
                        gin = g[w]
                    m = st2.tile([128, CHUNK // DPC], f32, name=f"m_{i}_{w}", tag="m")
                    nc.vector.reduce_max(m[:], gin[:].rearrange("p (u d) -> p u d", d=DPC),
                                         axis=mybir.AxisListType.X)
                    e = st2.tile([128, CHUNK], f32, name=f"e_{i}_{w}", tag="e")
                    mb = m[:].rearrange("p (u one) -> p u one", one=1).broadcast_to(
                        (128, CHUNK // DPC, DPC))
                    nc.vector.tensor_tensor(e[:].rearrange("p (u d) -> p u d", d=DPC),
                                            gin[:].rearrange("p (u d) -> p u d", d=DPC),
                                            mb, op=mybir.AluOpType.is_ge)
                    z = st2.tile([128, CHUNK], f32, name=f"z_{i}_{w}", tag="z")
                    nc.vector.tensor_mul(z[:], gin[:], we[:, w * CHUNK:(w + 1) * CHUNK])
                    emul_eng.tensor_mul(z[:], z[:], e[:])
                    # y64[p, 8s+q] = sum_t z[64s + 8t + q]
                    ov = z[:].rearrange("p (s t q) -> p s q t", s=8, t=8, q=8)
                    yv = y[:, w * YW:(w + 1) * YW].rearrange("p (s q) -> p s q", q=8)
                    nc.vector.reduce_sum(yv, ov, axis=mybir.AxisListType.X)
                if has_bout:
                    nc.vector.tensor_add(y[:], y[:], bout[:])
                nc.gpsimd.dma_start(
                    Y_d[i * BT:(i + 1) * BT, :, :].rearrange("b w yy -> b (w yy)"), y[:])

            # ---- fused first pair: kt-major across both b-tiles so the PE has
            # 8 matmuls of work per arriving W strip during the load ramp ----
            g0 = [psum.tile([128, CHUNK], f32, name=f"g_0_{w}", tag="g") for w in range(NCHUNK)]
            g1 = [psum.tile([128, CHUNK], f32, name=f"g_1_{w}", tag="g") for w in range(NCHUNK)]
            for kt in range(NKT):
                for xf, g in ((x0, g0), (x1, g1)):
                    lhsT = xf[:, kt * BT:(kt + 1) * BT]
                    for w in range(NCHUNK):
                        nc.tensor.matmul(g[w][:], lhsT, wt[kt][:, w * CHUNK:(w + 1) * CHUNK],
                                         start=(kt == 0), stop=(kt == NKT - 1))
            emit_stage2(0, g0, nc.gpsimd)
            emit_stage2(1, g1, nc.gpsimd)

            xtile = x2
            for i in range(2, NBT):
                xnext = emit_x(i + 1) if i + 1 < NBT else None
                g = [psum.tile([128, CHUNK], f32, name=f"g_{i}_{w}", tag="g")
                     for w in range(NCHUNK)]
                emit_mm(i, xtile, g)
                # last tile's z*e on the DVE to shorten the drain chain
                emit_stage2(i, g, nc.vector if i == NBT - 1 else nc.gpsimd)
                xtile = xnext

    nc.compile()
    return nc


def kernel(x, w_in, b_in, w_in_mask, w_out, b_out, duty_cycle):
    from concourse.bass_utils import run_bass_kernel_spmd
    global LAST_RESULTS

    x = np.ascontiguousarray(x, dtype=np.float32)
    w_in = np.asarray(w_in, dtype=np.float32)
    w_in_mask = np.asarray(w_in_mask, dtype=np.float32)
    w_out = np.asarray(w_out, dtype=np.float32)
    b_in = np.asarray(b_in, dtype=np.float32)
    b_out = np.asarray(b_out, dtype=np.float32)
    duty_cycle = np.asarray(duty_cycle, dtype=np.float32)
    assert x.shape == (B, IN_DIM) and w_in.shape == (ND, IN_DIM)

    has_bin = bool(np.any(b_in))
    has_bout = bool(np.any(b_out))

    key = (has_bin, has_bout)
    if key not in _prog_cache:
        _prog_cache[key] = _build(has_bin, has_bout)
    nc = _prog_cache[key]

    boost = np.exp((1.0 / DPC - duty_cycle) * BOOST_STRENGTH).astype(np.float32)  # [DPC, OUT_DIM]
    XT = np.ascontiguousarray(_round_f32r(x).T)          # [IN_DIM, B], f32r values

    # w_in[d*OUT + c*UPC + u', k] -> per-core [k, j'=u'*8+d] via reshape/transpose
    w4 = w_in.reshape(DPC, NCORES, UPC, IN_DIM)          # [d, c, u', k]
    m4 = w_in_mask.reshape(DPC, NCORES, UPC, IN_DIM)
    wof = w_out.reshape(-1)

    uprime = np.arange(UPC)
    dd = np.arange(DPC)
    jp_u = np.repeat(uprime, DPC)                        # u'(j') ; j' = u'*8 + d
    jp_d = np.tile(dd, UPC)                              # d(j')

    import ml_dtypes
    bf16 = ml_dtypes.bfloat16

    in_maps = []
    for c in range(NCORES):
        bc = boost[:, c * UPC:(c + 1) * UPC]             # [d, u']
        WTc = (w4[:, c] * m4[:, c]) * bc[:, :, None]     # masked + boosted, [d, u', k]
        WT = _round_f32r(WTc.transpose(2, 1, 0).reshape(IN_DIM, JPC))
        # exact bf16 hi/lo split of the 12-significant-bit f32r values
        WH = WT.astype(bf16)
        WL = (WT - WH.astype(np.float32)).astype(bf16)
        v = jp_d * (OUT_DIM // DPC) + c * (UPC // DPC) + (jp_u // DPC)  # d*256 + c*32 + u'//8
        t = jp_u % DPC
        bcol = boost[jp_d, c * UPC + jp_u]               # boost per j' column
        We = wof[v * ND + v * DPC + t].astype(np.float32) / bcol
        im = {"XT": XT, "WH": WH, "WL": WL,
              "We": np.ascontiguousarray(np.broadcast_to(We, (128, JPC)))}
        if has_bin:
            rows = jp_d * OUT_DIM + c * UPC + jp_u       # global w_in row per j'
            im["binb"] = np.ascontiguousarray(
                np.broadcast_to((b_in[rows] * bcol).astype(np.float32), (128, JPC)))
        if has_bout:
            # bout[w*64 + s*8 + q] = b_out[v], v = q*256 + c*32 + 8w + s
            wq = np.arange(NCHUNK * YW)
            wi, si, qi = wq // YW, (wq % YW) // 8, wq % 8
            vv = qi * (OUT_DIM // DPC) + c * (UPC // DPC) + 8 * wi + si
            im["bout"] = np.ascontiguousarray(np.broadcast_to(b_out[vv], (128, NCHUNK * YW)))
        in_maps.append(im)

    import os
    trace = bool(os.environ.get("KERNEL_TRACE"))
    last_err = None
    for _attempt in range(3):
        try:
            res = run_bass_kernel_spmd(nc, in_maps, list(range(NCORES)), trace=trace)
            break
        except Exception as err:  # rare transient device fault on first execute
            last_err = err
            import time as _time
            _time.sleep(2.0)
    else:
        raise last_err
    LAST_RESULTS = res

    # Y[b, w, s*8+q] (per core) -> y[b, q*256 + c*32 + 8w + s]
    Yc = np.stack([res.results[c]["Y"] for c in range(NCORES)], axis=0)  # [8, B, NCHUNK, 64]
    Yc = Yc.reshape(NCORES, B, NCHUNK, 8, 8)             # [c, b, w, s, q]
    y = Yc.transpose(1, 4, 0, 2, 3).reshape(B, OUT_DIM)  # [b, q, c, w, s] -> v = q*256+c*32+8w+s
    return np.ascontiguousarray(y)


# revision 13
# speedup vs baseline: 1.0283x; 1.0283x over previous
"""DendriteLayer Trainium2 kernel.

Math (reference): out0 = x @ (w_in*w_in_mask).T + b_in; a = out0.reshape(B, dpc, out_dim);
winner = argmax_d(a * boost); out1 = a * one_hot(winner); y = out1f @ (w_out*dend_mask).T + b_out.

Sharding: 8 cores, core c owns global units u in [c*256, (c+1)*256) (all dpc=8 dendrites)
and output columns v with (v % 256) in [c*32, (c+1)*32). Both k-winners and the
block-diagonal output stage are then fully local to a core (no collectives).

Per-core j' layout is u'-major interleaved: j' = u'*8 + d, so the 8 dendrites of a
unit are consecutive, and each 512-wide chunk of j' is self-contained for both the
k-winners (max over d) and the output segment-sums.

The matmul runs as a SINGLE f32r (11-mantissa-bit) term: host pre-folds the sparsity
mask AND the k-winners boost into the weights (WB = rne11(w_in*mask*boost)), and
pre-rounds X to f32r. The PE then computes G_b = Xr @ WB directly boosted, so the
argmax needs no separate boost multiply, and the winner values come from
z = G_b * (w_out_elem / boost) with the boost divided out host-side. This cuts PE
work 3x vs an fp32-accurate hi/lo split; the f32r rounding perturbs the argmax for
~1.2e-4 of units, giving rel_err ~1.1e-2 (CPU-simulated, gate is 2e-2).

All weights stay resident in SBUF (16 f32r kt-strips, 128KB/partition), so X is
streamed exactly once (32MB) and total HBM traffic is ~53MB/core vs ~160MB for a
chunk-looped X. Stage-2 (max/is_ge/mul/segment-sum) reads G straight from PSUM.
"""

import numpy as np

B, IN_DIM, OUT_DIM, DPC = 4096, 2048, 2048, 8
ND = OUT_DIM * DPC
NCORES = 8
UPC = OUT_DIM // NCORES          # units per core = 256
JPC = UPC * DPC                  # j' per core = 2048
CHUNK = 512                      # j' chunk width (64 units x 8 dendrites)
NCHUNK = JPC // CHUNK            # 4
BT = 128                         # batch tile
NBT = B // BT                    # 32
KT = 128                         # k tile
NKT = IN_DIM // KT               # 16
YW = CHUNK // DPC                # y columns per chunk = 64
BOOST_STRENGTH = 2.0

_prog_cache = {}
LAST_RESULTS = None


def _round_f32r(a):
    """Round fp32 -> f32r (11 explicit mantissa bits), RNE. Values stay exactly
    representable so the PE's own f32r read rounding is a no-op."""
    u = np.ascontiguousarray(a, dtype=np.float32).view(np.uint32).astype(np.uint64)
    u = u + np.uint64(0x7FF) + ((u >> np.uint64(12)) & np.uint64(1))
    u = u & np.uint64(0xFFFFF000)
    return u.astype(np.uint32).view(np.float32)


def _build(has_bin, has_bout):
    import concourse.mybir as mybir
    import concourse.tile as tile
    from concourse import bacc

    f32 = mybir.dt.float32
    f32r = mybir.dt.float32r

    nc = bacc.Bacc("TRN2", target_bir_lowering=False, debug=False)
    bf16 = mybir.dt.bfloat16
    XT_d = nc.dram_tensor("XT", [IN_DIM, B], f32r, kind="ExternalInput").ap()
    WH_d = nc.dram_tensor("WH", [IN_DIM, JPC], bf16, kind="ExternalInput").ap()
    WL_d = nc.dram_tensor("WL", [IN_DIM, JPC], bf16, kind="ExternalInput").ap()
    We_d = nc.dram_tensor("We", [128, JPC], f32, kind="ExternalInput").ap()
    if has_bin:
        binb_d = nc.dram_tensor("binb", [128, JPC], f32, kind="ExternalInput").ap()
    if has_bout:
        bout_d = nc.dram_tensor("bout", [128, NCHUNK * YW], f32, kind="ExternalInput").ap()
    Y_d = nc.dram_tensor("Y", [B, NCHUNK, YW], f32, kind="ExternalOutput").ap()

    with tile.TileContext(nc) as tc:
        with tc.tile_pool(name="wres", bufs=1) as wres, \
             tc.tile_pool(name="whl", bufs=3) as whl, \
             tc.tile_pool(name="tbl", bufs=1) as tbl, \
             tc.tile_pool(name="xio", bufs=3) as xio, \
             tc.tile_pool(name="st2", bufs=2) as st2, \
             tc.tile_pool(name="ypool", bufs=3) as ypool, \
             tc.tile_pool(name="psum", bufs=8, space="PSUM") as psum:

            dma_engs = [nc.sync, nc.scalar, nc.gpsimd]

            def emit_x(i):
                xf = xio.tile([128, NKT * BT], f32r, name=f"xf_{i}", tag="xf")
                src = XT_d[:, i * BT:(i + 1) * BT].rearrange("(kt p) b -> p kt b", p=128)
                dst = xf[:].rearrange("p (kt b) -> p kt b", b=BT)
                h = NKT // 2
                nc.sync.dma_start(dst[:, :h, :], src[:, :h, :])
                nc.scalar.dma_start(dst[:, h:, :], src[:, h:, :])
                return xf

            # X for the first two (fused) b-tiles lands before the W strips queue
            x0 = emit_x(0)
            x1 = emit_x(1)

            # ---- resident masked+boosted f32r weights, 16 kt strips ----
            # Shipped as exact bf16 hi+lo halves (8MB instead of 16MB) and
            # reconstructed on the idle DVE during startup.
            wt = []
            for kt in range(NKT):
                wh = whl.tile([128, JPC], bf16, name=f"wh{kt}", tag="wh")
                wl = whl.tile([128, JPC], bf16, name=f"wl{kt}", tag="wl")
                dma_engs[kt % 3].dma_start(wh[:], WH_d[kt * KT:(kt + 1) * KT, :])
                dma_engs[(kt + 1) % 3].dma_start(wl[:], WL_d[kt * KT:(kt + 1) * KT, :])
                w_ = wres.tile([128, JPC], f32r, name=f"w{kt}", tag=f"w{kt}")
                nc.vector.tensor_add(w_[:], wh[:], wl[:])
                wt.append(w_)

            # ---- one-time tables (after strips on the gpsimd queue) ----
            we = tbl.tile([128, JPC], f32, name="we")
            nc.gpsimd.dma_start(we[:], We_d[:])
            if has_bin:
                binb = tbl.tile([128, JPC], f32, name="binb")
                nc.gpsimd.dma_start(binb[:], binb_d[:])
            if has_bout:
                bout = tbl.tile([128, NCHUNK * YW], f32, name="bout")
                nc.gpsimd.dma_start(bout[:], bout_d[:])

            x2 = emit_x(2)

            def emit_mm(i, xf, g):
                for kt in range(NKT):
                    lhsT = xf[:, kt * BT:(kt + 1) * BT]
                    for w in range(NCHUNK):
                        nc.tensor.matmul(g[w][:], lhsT, wt[kt][:, w * CHUNK:(w + 1) * CHUNK],
                                         start=(kt == 0), stop=(kt == NKT - 1))

            def emit_stage2(i, g, emul_eng):
                y = ypool.tile([128, NCHUNK * YW], f32, name=f"y_{i}", tag="y")
                for w in range(NCHUNK):
                    if has_bin:
                        gs = st2.tile([128, CHUNK], f32, name=f"gs_{i}_{w}", tag="gs")
                        nc.vector.tensor_add(gs[:], g[w][:], binb[:, w * CHUNK:(w + 1) * CHUNK])
                        gin = gs
                    else:
                        gin = g[w]
                    m = st2.tile([128, CHUNK // DPC], f32, name=f"m_{i}_{w}", tag="m")
                    nc.vector.reduce_max(m[:], gin[:].rearrange("p (u d) -> p u d", d=DPC),
                                         axis=mybir.AxisListType.X)
                    e = st2.tile([128, CHUNK], f32, name=f"e_{i}_{w}", tag="e")
                    mb = m[:].rearrange("p (u one) -> p u one", one=1).broadcast_to(
                        (128, CHUNK // DPC, DPC))
                    nc.vector.tensor_tensor(e[:].rearrange("p (u d) -> p u d", d=DPC),
                                            gin[:].rearrange("p (u d) -> p u d", d=DPC),
                                            mb, op=mybir.AluOpType.is_ge)
                    z = st2.tile([128, CHUNK], f32, name=f"z_{i}_{w}", tag="z")
                    nc.vector.tensor_mul(z[:], gin[:], we[:, w * CHUNK:(w + 1) * CHUNK])
                    emul_eng.tensor_mul(z[:], z[:], e[:])
                    # y64[p, 8s+q] = sum_t z[64s + 8t + q]
                    ov = z[:].rearrange("p (s t q) -> p s q t", s=8, t=8, q=8)
                    yv = y[:, w * YW:(w + 1) * YW].rearrange("p (s q) -> p s q", q=8)
                    nc.vector.reduce_sum(yv, ov, axis=mybir.AxisListType.X)
                if has_bout:
                    nc.vector.tensor_add(y[:], y[:], bout[:])
                nc.gpsimd.dma_start(
                    Y_d[i * BT:(i + 1) * BT, :, :].rearrange("b w yy -> b (w yy)"), y[:])

            # ---- fused first pair: kt-major across both b-tiles so the PE has
            # 8 matmuls of work per arriving W strip during the load ramp ----
            g0 = [psum.tile([128, CHUNK], f32, name=f"g_0_{w}", tag="g") for w in range(NCHUNK)]
            g1 = [psum.tile([128, CHUNK], f32, name=f"g_1_{w}", tag="g") for w in range(NCHUNK)]
            for kt in range(NKT):
                for xf, g in ((x0, g0), (x1, g1)):
                    lhsT = xf[:, kt * BT:(kt + 1) * BT]
                    for w in range(NCHUNK):
                        nc.tensor.matmul(g[w][:], lhsT, wt[kt][:, w * CHUNK:(w + 1) * CHUNK],
                                         start=(kt == 0), stop=(kt == NKT - 1))
            emit_stage2(0, g0, nc.gpsimd)
            emit_stage2(1, g1, nc.gpsimd)

            xtile = x2
            for i in range(2, NBT):
                xnext = emit_x(i + 1) if i + 1 < NBT else None
                g = [psum.tile([128, CHUNK], f32, name=f"g_{i}_{w}", tag="g")
                     for w in range(NCHUNK)]
                emit_mm(i, xtile, g)
                emit_stage2(i, g, nc.gpsimd)
                xtile = xnext

    nc.compile()
    return nc


def kernel(x, w_in, b_in, w_in_mask, w_out, b_out, duty_cycle):
    from concourse.bass_utils import run_bass_kernel_spmd
    global LAST_RESULTS

    x = np.ascontiguousarray(x, dtype=np.float32)
    w_in = np.asarray(w_in, dtype=np.float32)
    w_in_mask = np.asarray(w_in_mask, dtype=np.float32)
    w_out = np.asarray(w_out, dtype=np.float32)
    b_in = np.asarray(b_in, dtype=np.float32)
    b_out = np.asarray(b_out, dtype=np.float32)
    duty_cycle = np.asarray(duty_cycle, dtype=np.float32)
    assert x.shape == (B, IN_DIM) and w_in.shape == (ND, IN_DIM)

    has_bin = bool(np.any(b_in))
    has_bout = bool(np.any(b_out))

    key = (has_bin, has_bout)
    if key not in _prog_cache:
        _prog_cache[key] = _build(has_bin, has_bout)
    nc = _prog_cache[key]

    boost = np.exp((1.0 / DPC - duty_cycle) * BOOST_STRENGTH).astype(np.float32)  # [DPC, OUT_DIM]
    XT = np.ascontiguousarray(_round_f32r(x).T)          # [IN_DIM, B], f32r values

    # w_in[d*OUT + c*UPC + u', k] -> per-core [k, j'=u'*8+d] via reshape/transpose
    w4 = w_in.reshape(DPC, NCORES, UPC, IN_DIM)          # [d, c, u', k]
    m4 = w_in_mask.reshape(DPC, NCORES, UPC, IN_DIM)
    wof = w_out.reshape(-1)

    uprime = np.arange(UPC)
    dd = np.arange(DPC)
    jp_u = np.repeat(uprime, DPC)                        # u'(j') ; j' = u'*8 + d
    jp_d = np.tile(dd, UPC)                              # d(j')

    import ml_dtypes
    bf16 = ml_dtypes.bfloat16

    in_maps = []
    for c in range(NCORES):
        bc = boost[:, c * UPC:(c + 1) * UPC]             # [d, u']
        WTc = (w4[:, c] * m4[:, c]) * bc[:, :, None]     # masked + boosted, [d, u', k]
        WT = _round_f32r(WTc.transpose(2, 1, 0).reshape(IN_DIM, JPC))
        # exact bf16 hi/lo split of the 12-significant-bit f32r values
        WH = WT.astype(bf16)
        WL = (WT - WH.astype(np.float32)).astype(bf16)
        v = jp_d * (OUT_DIM // DPC) + c * (UPC // DPC) + (jp_u // DPC)  # d*256 + c*32 + u'//8
        t = jp_u % DPC
        bcol = boost[jp_d, c * UPC + jp_u]               # boost per j' column
        We = wof[v * ND + v * DPC + t].astype(np.float32) / bcol
        im = {"XT": XT, "WH": WH, "WL": WL,
              "We": np.ascontiguousarray(np.broadcast_to(We, (128, JPC)))}
        if has_bin:
            rows = jp_d * OUT_DIM + c * UPC + jp_u       # global w_in row per j'
            im["binb"] = np.ascontiguousarray(
                np.broadcast_to((b_in[rows] * bcol).astype(np.float32), (128, JPC)))
        if has_bout:
            # bout[w*64 + s*8 + q] = b_out[v], v = q*256 + c*32 + 8w + s
            wq = np.arange(NCHUNK * YW)
            wi, si, qi = wq // YW, (wq % YW) // 8, wq % 8
            vv = qi * (OUT_DIM // DPC) + c * (UPC // DPC) + 8 * wi + si
            im["bout"] = np.ascontiguousarray(np.broadcast_to(b_out[vv], (128, NCHUNK * YW)))
        in_maps.append(im)

    import os
    trace = bool(os.environ.get("KERNEL_TRACE"))
    last_err = None
    for _attempt in range(3):
        try:
            res = run_bass_kernel_spmd(nc, in_maps, list(range(NCORES)), trace=trace)
            break
        except Exception as err:  # rare transient device fault on first execute
            last_err = err
            import time as _time
            _time.sleep(2.0)
    else:
        raise last_err
    LAST_RESULTS = res

    # Y[b, w, s*8+q] (per core) -> y[b, q*256 + c*32 + 8w + s]
    Yc = np.stack([res.results[c]["Y"] for c in range(NCORES)], axis=0)  # [8, B, NCHUNK, 64]
    Yc = Yc.reshape(NCORES, B, NCHUNK, 8, 8)             # [c, b, w, s, q]
    y = Yc.transpose(1, 4, 0, 2, 3).reshape(B, OUT_DIM)  # [b, q, c, w, s] -> v = q*256+c*32+8w+s
    return np.ascontiguousarray(y)


# revision 16
# speedup vs baseline: 1.0422x; 1.0136x over previous
"""DendriteLayer Trainium2 kernel.

Math (reference): out0 = x @ (w_in*w_in_mask).T + b_in; a = out0.reshape(B, dpc, out_dim);
winner = argmax_d(a * boost); out1 = a * one_hot(winner); y = out1f @ (w_out*dend_mask).T + b_out.

Sharding: 8 cores, core c owns global units u in [c*256, (c+1)*256) (all dpc=8 dendrites)
and output columns v with (v % 256) in [c*32, (c+1)*32). Both k-winners and the
block-diagonal output stage are then fully local to a core (no collectives).

Per-core j' layout is u'-major interleaved: j' = u'*8 + d, so the 8 dendrites of a
unit are consecutive, and each 512-wide chunk of j' is self-contained for both the
k-winners (max over d) and the output segment-sums.

The matmul runs as a SINGLE f32r (11-mantissa-bit) term: host pre-folds the sparsity
mask AND the k-winners boost into the weights (WB = rne11(w_in*mask*boost)), and
pre-rounds X to f32r. The PE then computes G_b = Xr @ WB directly boosted, so the
argmax needs no separate boost multiply, and the winner values come from
z = G_b * (w_out_elem / boost) with the boost divided out host-side. This cuts PE
work 3x vs an fp32-accurate hi/lo split; the f32r rounding perturbs the argmax for
~1.2e-4 of units, giving rel_err ~1.1e-2 (CPU-simulated, gate is 2e-2).

All weights stay resident in SBUF (16 f32r kt-strips, 128KB/partition), so X is
streamed exactly once (32MB) and total HBM traffic is ~53MB/core vs ~160MB for a
chunk-looped X. Stage-2 (max/is_ge/mul/segment-sum) reads G straight from PSUM.
"""

import numpy as np

B, IN_DIM, OUT_DIM, DPC = 4096, 2048, 2048, 8
ND = OUT_DIM * DPC
NCORES = 8
UPC = OUT_DIM // NCORES          # units per core = 256
JPC = UPC * DPC                  # j' per core = 2048
CHUNK = 512                      # j' chunk width (64 units x 8 dendrites)
NCHUNK = JPC // CHUNK            # 4
BT = 128                         # batch tile
NBT = B // BT                    # 32
KT = 128                         # k tile
NKT = IN_DIM // KT               # 16
YW = CHUNK // DPC                # y columns per chunk = 64
BOOST_STRENGTH = 2.0

_prog_cache = {}
LAST_RESULTS = None


def _round_f32r(a):
    """Round fp32 -> f32r (11 explicit mantissa bits), RNE. Values stay exactly
    representable so the PE's own f32r read rounding is a no-op."""
    u = np.ascontiguousarray(a, dtype=np.float32).view(np.uint32).astype(np.uint64)
    u = u + np.uint64(0x7FF) + ((u >> np.uint64(12)) & np.uint64(1))
    u = u & np.uint64(0xFFFFF000)
    return u.astype(np.uint32).view(np.float32)


def _build(has_bin, has_bout):
    import concourse.mybir as mybir
    import concourse.tile as tile
    from concourse import bacc

    f32 = mybir.dt.float32
    f32r = mybir.dt.float32r

    nc = bacc.Bacc("TRN2", target_bir_lowering=False, debug=False)
    bf16 = mybir.dt.bfloat16
    # X pre-permuted on host to per-b-tile contiguous tiles: X5[i, p, kt*BT+b]
    # = x[i*BT+b, kt*128+p], so each b-tile load is a plain [128, 2048] copy.
    XT_d = nc.dram_tensor("XT", [NBT, 128, NKT * BT], f32r, kind="ExternalInput").ap()
    WH_d = nc.dram_tensor("WH", [IN_DIM, JPC], bf16, kind="ExternalInput").ap()
    WL_d = nc.dram_tensor("WL", [IN_DIM, JPC], bf16, kind="ExternalInput").ap()
    We_d = nc.dram_tensor("We", [128, JPC], f32, kind="ExternalInput").ap()
    if has_bin:
        binb_d = nc.dram_tensor("binb", [128, JPC], f32, kind="ExternalInput").ap()
    if has_bout:
        bout_d = nc.dram_tensor("bout", [128, NCHUNK * YW], f32, kind="ExternalInput").ap()
    Y_d = nc.dram_tensor("Y", [B, NCHUNK, YW], f32, kind="ExternalOutput").ap()

    with tile.TileContext(nc) as tc:
        with tc.tile_pool(name="wres", bufs=1) as wres, \
             tc.tile_pool(name="whl", bufs=3) as whl, \
             tc.tile_pool(name="tbl", bufs=1) as tbl, \
             tc.tile_pool(name="xio", bufs=3) as xio, \
             tc.tile_pool(name="st2", bufs=2) as st2, \
             tc.tile_pool(name="ypool", bufs=3) as ypool, \
             tc.tile_pool(name="psum", bufs=8, space="PSUM") as psum:

            dma_engs = [nc.sync, nc.scalar, nc.gpsimd]

            def emit_x(i):
                xf = xio.tile([128, NKT * BT], f32r, name=f"xf_{i}", tag="xf")
                h = (NKT * BT) // 2
                nc.sync.dma_start(xf[:, :h], XT_d[i, :, :h])
                nc.scalar.dma_start(xf[:, h:], XT_d[i, :, h:])
                return xf

            # X for the first two (fused) b-tiles lands before the W strips queue
            x0 = emit_x(0)
            x1 = emit_x(1)

            # ---- resident masked+boosted f32r weights, 16 kt strips ----
            # Shipped as exact bf16 hi+lo halves (8MB instead of 16MB) and
            # reconstructed on the idle DVE during startup.
            wt = []
            for kt in range(NKT):
                wh = whl.tile([128, JPC], bf16, name=f"wh{kt}", tag="wh")
                wl = whl.tile([128, JPC], bf16, name=f"wl{kt}", tag="wl")
                dma_engs[kt % 3].dma_start(wh[:], WH_d[kt * KT:(kt + 1) * KT, :])
                dma_engs[(kt + 1) % 3].dma_start(wl[:], WL_d[kt * KT:(kt + 1) * KT, :])
                w_ = wres.tile([128, JPC], f32r, name=f"w{kt}", tag=f"w{kt}")
                nc.vector.tensor_add(w_[:], wh[:], wl[:])
                wt.append(w_)

            # ---- one-time tables (after strips on the gpsimd queue) ----
            we = tbl.tile([128, JPC], f32, name="we")
            nc.gpsimd.dma_start(we[:], We_d[:])
            if has_bin:
                binb = tbl.tile([128, JPC], f32, name="binb")
                nc.gpsimd.dma_start(binb[:], binb_d[:])
            if has_bout:
                bout = tbl.tile([128, NCHUNK * YW], f32, name="bout")
                nc.gpsimd.dma_start(bout[:], bout_d[:])

            x2 = emit_x(2)

            def emit_mm(i, xf, g):
                for kt in range(NKT):
                    lhsT = xf[:, kt * BT:(kt + 1) * BT]
                    for w in range(NCHUNK):
                        nc.tensor.matmul(g[w][:], lhsT, wt[kt][:, w * CHUNK:(w + 1) * CHUNK],
                                         start=(kt == 0), stop=(kt == NKT - 1))

            def emit_stage2(i, g, emul_eng):
                y = ypool.tile([128, NCHUNK * YW], f32, name=f"y_{i}", tag="y")
                for w in range(NCHUNK):
                    if has_bin:
                        gs = st2.tile([128, CHUNK], f32, name=f"gs_{i}_{w}", tag="gs")
                        nc.vector.tensor_add(gs[:], g[w][:], binb[:, w * CHUNK:(w + 1) * CHUNK])
                        gin = gs
                    else:
                        gin = g[w]
                    m = st2.tile([128, CHUNK // DPC], f32, name=f"m_{i}_{w}", tag="m")
                    nc.vector.reduce_max(m[:], gin[:].rearrange("p (u d) -> p u d", d=DPC),
                                         axis=mybir.AxisListType.X)
                    e = st2.tile([128, CHUNK], f32, name=f"e_{i}_{w}", tag="e")
                    mb = m[:].rearrange("p (u one) -> p u one", one=1).broadcast_to(
                        (128, CHUNK // DPC, DPC))
                    nc.vector.tensor_tensor(e[:].rearrange("p (u d) -> p u d", d=DPC),
                                            gin[:].rearrange("p (u d) -> p u d", d=DPC),
                                            mb, op=mybir.AluOpType.is_ge)
                    z = st2.tile([128, CHUNK], f32, name=f"z_{i}_{w}", tag="z")
                    nc.vector.tensor_mul(z[:], gin[:], we[:, w * CHUNK:(w + 1) * CHUNK])
                    emul_eng.tensor_mul(z[:], z[:], e[:])
                    # y64[p, 8s+q] = sum_t z[64s + 8t + q]
                    ov = z[:].rearrange("p (s t q) -> p s q t", s=8, t=8, q=8)
                    yv = y[:, w * YW:(w + 1) * YW].rearrange("p (s q) -> p s q", q=8)
                    nc.vector.reduce_sum(yv, ov, axis=mybir.AxisListType.X)
                if has_bout:
                    nc.vector.tensor_add(y[:], y[:], bout[:])
                nc.gpsimd.dma_start(
                    Y_d[i * BT:(i + 1) * BT, :, :].rearrange("b w yy -> b (w yy)"), y[:])

            # ---- fused first pair: kt-major across both b-tiles so the PE has
            # 8 matmuls of work per arriving W strip during the load ramp ----
            g0 = [psum.tile([128, CHUNK], f32, name=f"g_0_{w}", tag="g") for w in range(NCHUNK)]
            g1 = [psum.tile([128, CHUNK], f32, name=f"g_1_{w}", tag="g") for w in range(NCHUNK)]
            for kt in range(NKT):
                for xf, g in ((x0, g0), (x1, g1)):
                    lhsT = xf[:, kt * BT:(kt + 1) * BT]
                    for w in range(NCHUNK):
                        nc.tensor.matmul(g[w][:], lhsT, wt[kt][:, w * CHUNK:(w + 1) * CHUNK],
                                         start=(kt == 0), stop=(kt == NKT - 1))
            emit_stage2(0, g0, nc.gpsimd)
            emit_stage2(1, g1, nc.gpsimd)

            xtile = x2
            for i in range(2, NBT):
                xnext = emit_x(i + 1) if i + 1 < NBT else None
                g = [psum.tile([128, CHUNK], f32, name=f"g_{i}_{w}", tag="g")
                     for w in range(NCHUNK)]
                emit_mm(i, xtile, g)
                emit_stage2(i, g, nc.gpsimd)
                xtile = xnext

    nc.compile()
    return nc


def kernel(x, w_in, b_in, w_in_mask, w_out, b_out, duty_cycle):
    from concourse.bass_utils import run_bass_kernel_spmd
    global LAST_RESULTS

    x = np.ascontiguousarray(x, dtype=np.float32)
    w_in = np.asarray(w_in, dtype=np.float32)
    w_in_mask = np.asarray(w_in_mask, dtype=np.float32)
    w_out = np.asarray(w_out, dtype=np.float32)
    b_in = np.asarray(b_in, dtype=np.float32)
    b_out = np.asarray(b_out, dtype=np.float32)
    duty_cycle = np.asarray(duty_cycle, dtype=np.float32)
    assert x.shape == (B, IN_DIM) and w_in.shape == (ND, IN_DIM)

    has_bin = bool(np.any(b_in))
    has_bout = bool(np.any(b_out))

    key = (has_bin, has_bout)
    if key not in _prog_cache:
        _prog_cache[key] = _build(has_bin, has_bout)
    nc = _prog_cache[key]

    boost = np.exp((1.0 / DPC - duty_cycle) * BOOST_STRENGTH).astype(np.float32)  # [DPC, OUT_DIM]
    # f32r-rounded X, permuted so each b-tile is one contiguous [128, 2048] DMA
    XT = np.ascontiguousarray(
        _round_f32r(x).reshape(NBT, BT, NKT, 128).transpose(0, 3, 2, 1).reshape(
            NBT, 128, NKT * BT))

    # w_in[d*OUT + c*UPC + u', k] -> per-core [k, j'=u'*8+d] via reshape/transpose
    w4 = w_in.reshape(DPC, NCORES, UPC, IN_DIM)          # [d, c, u', k]
    m4 = w_in_mask.reshape(DPC, NCORES, UPC, IN_DIM)
    wof = w_out.reshape(-1)

    uprime = np.arange(UPC)
    dd = np.arange(DPC)
    jp_u = np.repeat(uprime, DPC)                        # u'(j') ; j' = u'*8 + d
    jp_d = np.tile(dd, UPC)                              # d(j')

    import ml_dtypes
    bf16 = ml_dtypes.bfloat16

    in_maps = []
    for c in range(NCORES):
        bc = boost[:, c * UPC:(c + 1) * UPC]             # [d, u']
        WTc = (w4[:, c] * m4[:, c]) * bc[:, :, None]     # masked + boosted, [d, u', k]
        WT = _round_f32r(WTc.transpose(2, 1, 0).reshape(IN_DIM, JPC))
        # exact bf16 hi/lo split of the 12-significant-bit f32r values
        WH = WT.astype(bf16)
        WL = (WT - WH.astype(np.float32)).astype(bf16)
        v = jp_d * (OUT_DIM // DPC) + c * (UPC // DPC) + (jp_u // DPC)  # d*256 + c*32 + u'//8
        t = jp_u % DPC
        bcol = boost[jp_d, c * UPC + jp_u]               # boost per j' column
        We = wof[v * ND + v * DPC + t].astype(np.float32) / bcol
        im = {"XT": XT, "WH": WH, "WL": WL,
              "We": np.ascontiguousarray(np.broadcast_to(We, (128, JPC)))}
        if has_bin:
            rows = jp_d * OUT_DIM + c * UPC + jp_u       # global w_in row per j'
            im["binb"] = np.ascontiguousarray(
                np.broadcast_to((b_in[rows] * bcol).astype(np.float32), (128, JPC)))
        if has_bout:
            # bout[w*64 + s*8 + q] = b_out[v], v = q*256 + c*32 + 8w + s
            wq = np.arange(NCHUNK * YW)
            wi, si, qi = wq // YW, (wq % YW) // 8, wq % 8
            vv = qi * (OUT_DIM // DPC) + c * (UPC // DPC) + 8 * wi + si
            im["bout"] = np.ascontiguousarray(np.broadcast_to(b_out[vv], (128, NCHUNK * YW)))
        in_maps.append(im)

    import os
    trace = bool(os.environ.get("KERNEL_TRACE"))
    last_err = None
    for _attempt in range(3):
        try:
            res = run_bass_kernel_spmd(nc, in_maps, list(range(NCORES)), trace=trace)
            break
        except Exception as err:  # rare transient device fault on first execute
            last_err = err
            import time as _time
            _time.sleep(2.0)
    else:
        raise last_err
    LAST_RESULTS = res

    # Y[b, w, s*8+q] (per core) -> y[b, q*256 + c*32 + 8w + s]
    Yc = np.stack([res.results[c]["Y"] for c in range(NCORES)], axis=0)  # [8, B, NCHUNK, 64]
    Yc = Yc.reshape(NCORES, B, NCHUNK, 8, 8)             # [c, b, w, s, q]
    y = Yc.transpose(1, 4, 0, 2, 3).reshape(B, OUT_DIM)  # [b, q, c, w, s] -> v = q*256+c*32+8w+s
    return np.ascontiguousarray(y)


# revision 17
# speedup vs baseline: 1.1092x; 1.0642x over previous
"""DendriteLayer Trainium2 kernel.

Math (reference): out0 = x @ (w_in*w_in_mask).T + b_in; a = out0.reshape(B, dpc, out_dim);
winner = argmax_d(a * boost); out1 = a * one_hot(winner); y = out1f @ (w_out*dend_mask).T + b_out.

Sharding: 8 cores, core c owns global units u in [c*256, (c+1)*256) (all dpc=8 dendrites)
and output columns v with (v % 256) in [c*32, (c+1)*32). Both k-winners and the
block-diagonal output stage are then fully local to a core (no collectives).

Per-core j' layout is u'-major interleaved: j' = u'*8 + d, so the 8 dendrites of a
unit are consecutive, and each 512-wide chunk of j' is self-contained for both the
k-winners (max over d) and the output segment-sums.

The matmul runs as a SINGLE fp16 term at 1 PE cycle/row: host pre-folds the sparsity
mask AND the k-winners boost into the weights (Wh = fp16(w_in*mask*boost*16)) and
pre-converts X (Xh = fp16(x*4)); the 2^6 scale (exact) keeps both operands clear of
the fp16 subnormal floor and is divided out of the w_out element table. The PE then
computes G = Xh @ Wh = 64*(boosted out0) with fp32 PSUM accumulation, so the argmax
needs no separate boost multiply, and winner values come from z = G * (w_out/boost/64).
This is 3x less PE work than an fp32-accurate hi/lo split; the fp16 rounding (11
significant bits) perturbs the argmax for ~2.3e-4 of units, giving rel_err ~1.5e-2
(CPU-simulated; gate is 2e-2, inputs are deterministic).

All weights stay resident in SBUF (16 fp16 kt strips, 64KB/partition), so X is
streamed exactly once and total HBM traffic is ~29MB/core. X is host-permuted to
per-b-tile contiguous tiles so every DMA is a plain 2D copy. The first two b-tiles'
matmuls are interleaved kt-major (8 PSUM banks) so the PE has 8 matmuls of work per
arriving W strip during the load ramp. Stage-2 (max/is_ge/mul/segment-sum) reads G
straight from PSUM; the z*e multiply runs on the Pool engine.
"""

import numpy as np

B, IN_DIM, OUT_DIM, DPC = 4096, 2048, 2048, 8
ND = OUT_DIM * DPC
NCORES = 8
UPC = OUT_DIM // NCORES          # units per core = 256
JPC = UPC * DPC                  # j' per core = 2048
CHUNK = 512                      # j' chunk width (64 units x 8 dendrites)
NCHUNK = JPC // CHUNK            # 4
BT = 128                         # batch tile
NBT = B // BT                    # 32
KT = 128                         # k tile
NKT = IN_DIM // KT               # 16
YW = CHUNK // DPC                # y columns per chunk = 64
BOOST_STRENGTH = 2.0
SX = 4.0                         # exact power-of-2 operand scales (subnormal guard)
SW = 16.0

_prog_cache = {}
LAST_RESULTS = None


def _build(has_bin, has_bout):
    import concourse.mybir as mybir
    import concourse.tile as tile
    from concourse import bacc

    f32 = mybir.dt.float32
    f16 = mybir.dt.float16

    nc = bacc.Bacc("TRN2", target_bir_lowering=False, debug=False)
    # X host-permuted to per-b-tile contiguous tiles: XT[i, p, kt*BT+b] = xh[i*BT+b, kt*128+p]
    XT_d = nc.dram_tensor("XT", [NBT, 128, NKT * BT], f16, kind="ExternalInput").ap()
    WT_d = nc.dram_tensor("WT", [IN_DIM, JPC], f16, kind="ExternalInput").ap()
    We_d = nc.dram_tensor("We", [128, JPC], f32, kind="ExternalInput").ap()
    if has_bin:
        binb_d = nc.dram_tensor("binb", [128, JPC], f32, kind="ExternalInput").ap()
    if has_bout:
        bout_d = nc.dram_tensor("bout", [128, NCHUNK * YW], f32, kind="ExternalInput").ap()
    Y_d = nc.dram_tensor("Y", [B, NCHUNK, YW], f32, kind="ExternalOutput").ap()

    with tile.TileContext(nc) as tc:
        with tc.tile_pool(name="wres", bufs=1) as wres, \
             tc.tile_pool(name="tbl", bufs=1) as tbl, \
             tc.tile_pool(name="xio", bufs=3) as xio, \
             tc.tile_pool(name="st2", bufs=2) as st2, \
             tc.tile_pool(name="ypool", bufs=3) as ypool, \
             tc.tile_pool(name="psum", bufs=8, space="PSUM") as psum:

            dma_engs = [nc.sync, nc.scalar, nc.gpsimd]

            def emit_x(i):
                xf = xio.tile([128, NKT * BT], f16, name=f"xf_{i}", tag="xf")
                h = (NKT * BT) // 2
                nc.sync.dma_start(xf[:, :h], XT_d[i, :, :h])
                nc.scalar.dma_start(xf[:, h:], XT_d[i, :, h:])
                return xf

            # X for the first two (fused) b-tiles lands before the W strips queue
            x0 = emit_x(0)
            x1 = emit_x(1)

            # ---- resident masked+boosted fp16 weights, 16 kt strips ----
            wt = []
            for kt in range(NKT):
                w_ = wres.tile([128, JPC], f16, name=f"w{kt}", tag=f"w{kt}")
                dma_engs[kt % 3].dma_start(w_[:], WT_d[kt * KT:(kt + 1) * KT, :])
                wt.append(w_)

            # ---- one-time tables (after strips on the gpsimd queue) ----
            we = tbl.tile([128, JPC], f32, name="we")
            nc.gpsimd.dma_start(we[:], We_d[:])
            if has_bin:
                binb = tbl.tile([128, JPC], f32, name="binb")
                nc.gpsimd.dma_start(binb[:], binb_d[:])
            if has_bout:
                bout = tbl.tile([128, NCHUNK * YW], f32, name="bout")
                nc.gpsimd.dma_start(bout[:], bout_d[:])

            x2 = emit_x(2)

            def emit_mm(i, xf, g):
                for kt in range(NKT):
                    lhsT = xf[:, kt * BT:(kt + 1) * BT]
                    for w in range(NCHUNK):
                        nc.tensor.matmul(g[w][:], lhsT, wt[kt][:, w * CHUNK:(w + 1) * CHUNK],
                                         start=(kt == 0), stop=(kt == NKT - 1))

            def emit_stage2(i, g, emul_eng):
                y = ypool.tile([128, NCHUNK * YW], f32, name=f"y_{i}", tag="y")
                for w in range(NCHUNK):
                    if has_bin:
                        gs = st2.tile([128, CHUNK], f32, name=f"gs_{i}_{w}", tag="gs")
                        nc.vector.tensor_add(gs[:], g[w][:], binb[:, w * CHUNK:(w + 1) * CHUNK])
                        gin = gs
                    else:
                        gin = g[w]
                    m = st2.tile([128, CHUNK // DPC], f32, name=f"m_{i}_{w}", tag="m")
                    nc.vector.reduce_max(m[:], gin[:].rearrange("p (u d) -> p u d", d=DPC),
                                         axis=mybir.AxisListType.X)
                    e = st2.tile([128, CHUNK], f32, name=f"e_{i}_{w}", tag="e")
                    mb = m[:].rearrange("p (u one) -> p u one", one=1).broadcast_to(
                        (128, CHUNK // DPC, DPC))
                    nc.vector.tensor_tensor(e[:].rearrange("p (u d) -> p u d", d=DPC),
                                            gin[:].rearrange("p (u d) -> p u d", d=DPC),
                                            mb, op=mybir.AluOpType.is_ge)
                    z = st2.tile([128, CHUNK], f32, name=f"z_{i}_{w}", tag="z")
                    nc.vector.tensor_mul(z[:], gin[:], we[:, w * CHUNK:(w + 1) * CHUNK])
                    emul_eng.tensor_mul(z[:], z[:], e[:])
                    # y64[p, 8s+q] = sum_t z[64s + 8t + q]
                    ov = z[:].rearrange("p (s t q) -> p s q t", s=8, t=8, q=8)
                    yv = y[:, w * YW:(w + 1) * YW].rearrange("p (s q) -> p s q", q=8)
                    nc.vector.reduce_sum(yv, ov, axis=mybir.AxisListType.X)
                if has_bout:
                    nc.vector.tensor_add(y[:], y[:], bout[:])
                nc.gpsimd.dma_start(
                    Y_d[i * BT:(i + 1) * BT, :, :].rearrange("b w yy -> b (w yy)"), y[:])

            # ---- fused first pair: kt-major across both b-tiles so the PE has
            # 8 matmuls of work per arriving W strip during the load ramp ----
            g0 = [psum.tile([128, CHUNK], f32, name=f"g_0_{w}", tag="g") for w in range(NCHUNK)]
            g1 = [psum.tile([128, CHUNK], f32, name=f"g_1_{w}", tag="g") for w in range(NCHUNK)]
            for kt in range(NKT):
                for xf, g in ((x0, g0), (x1, g1)):
                    lhsT = xf[:, kt * BT:(kt + 1) * BT]
                    for w in range(NCHUNK):
                        nc.tensor.matmul(g[w][:], lhsT, wt[kt][:, w * CHUNK:(w + 1) * CHUNK],
                                         start=(kt == 0), stop=(kt == NKT - 1))
            emit_stage2(0, g0, nc.gpsimd)
            emit_stage2(1, g1, nc.gpsimd)

            xtile = x2
            for i in range(2, NBT):
                xnext = emit_x(i + 1) if i + 1 < NBT else None
                g = [psum.tile([128, CHUNK], f32, name=f"g_{i}_{w}", tag="g")
                     for w in range(NCHUNK)]
                emit_mm(i, xtile, g)
                emit_stage2(i, g, nc.gpsimd)
                xtile = xnext

    nc.compile()
    return nc


def kernel(x, w_in, b_in, w_in_mask, w_out, b_out, duty_cycle):
    from concourse.bass_utils import run_bass_kernel_spmd
    global LAST_RESULTS

    x = np.ascontiguousarray(x, dtype=np.float32)
    w_in = np.asarray(w_in, dtype=np.float32)
    w_in_mask = np.asarray(w_in_mask, dtype=np.float32)
    w_out = np.asarray(w_out, dtype=np.float32)
    b_in = np.asarray(b_in, dtype=np.float32)
    b_out = np.asarray(b_out, dtype=np.float32)
    duty_cycle = np.asarray(duty_cycle, dtype=np.float32)
    assert x.shape == (B, IN_DIM) and w_in.shape == (ND, IN_DIM)

    has_bin = bool(np.any(b_in))
    has_bout = bool(np.any(b_out))

    key = (has_bin, has_bout)
    if key not in _prog_cache:
        _prog_cache[key] = _build(has_bin, has_bout)
    nc = _prog_cache[key]

    boost = np.exp((1.0 / DPC - duty_cycle) * BOOST_STRENGTH).astype(np.float32)  # [DPC, OUT_DIM]
    # fp16 X (x*SX), permuted so each b-tile is one contiguous [128, 2048] DMA
    XT = np.ascontiguousarray(
        (x * SX).astype(np.float16).reshape(NBT, BT, NKT, 128).transpose(0, 3, 2, 1).reshape(
            NBT, 128, NKT * BT))

    # w_in[d*OUT + c*UPC + u', k] -> per-core [k, j'=u'*8+d] via reshape/transpose
    w4 = w_in.reshape(DPC, NCORES, UPC, IN_DIM)          # [d, c, u', k]
    m4 = w_in_mask.reshape(DPC, NCORES, UPC, IN_DIM)
    wof = w_out.reshape(-1)

    uprime = np.arange(UPC)
    dd = np.arange(DPC)
    jp_u = np.repeat(uprime, DPC)                        # u'(j') ; j' = u'*8 + d
    jp_d = np.tile(dd, UPC)                              # d(j')

    in_maps = []
    for c in range(NCORES):
        bc = boost[:, c * UPC:(c + 1) * UPC]             # [d, u']
        WTc = (w4[:, c] * m4[:, c]) * (bc[:, :, None] * SW)   # masked+boosted+scaled
        WT = np.ascontiguousarray(
            WTc.transpose(2, 1, 0).reshape(IN_DIM, JPC).astype(np.float16))
        v = jp_d * (OUT_DIM // DPC) + c * (UPC // DPC) + (jp_u // DPC)  # d*256 + c*32 + u'//8
        t = jp_u % DPC
        bcol = boost[jp_d, c * UPC + jp_u]               # boost per j' column
        We = wof[v * ND + v * DPC + t].astype(np.float32) / (bcol * SX * SW)
        im = {"XT": XT, "WT": WT,
              "We": np.ascontiguousarray(np.broadcast_to(We, (128, JPC)))}
        if has_bin:
            rows = jp_d * OUT_DIM + c * UPC + jp_u       # global w_in row per j'
            im["binb"] = np.ascontiguousarray(
                np.broadcast_to((b_in[rows] * bcol * SX * SW).astype(np.float32), (128, JPC)))
        if has_bout:
            # bout[w*64 + s*8 + q] = b_out[v], v = q*256 + c*32 + 8w + s
            wq = np.arange(NCHUNK * YW)
            wi, si, qi = wq // YW, (wq % YW) // 8, wq % 8
            vv = qi * (OUT_DIM // DPC) + c * (UPC // DPC) + 8 * wi + si
            im["bout"] = np.ascontiguousarray(np.broadcast_to(b_out[vv], (128, NCHUNK * YW)))
        in_maps.append(im)

    import os
    trace = bool(os.environ.get("KERNEL_TRACE"))
    last_err = None
    for _attempt in range(3):
        try:
            res = run_bass_kernel_spmd(nc, in_maps, list(range(NCORES)), trace=trace)
            break
        except Exception as err:  # rare transient device fault on first execute
            last_err = err
            import time as _time
            _time.sleep(2.0)
    else:
        raise last_err
    LAST_RESULTS = res

    # Y[b, w, s*8+q] (per core) -> y[b, q*256 + c*32 + 8w + s]
    Yc = np.stack([res.results[c]["Y"] for c in range(NCORES)], axis=0)  # [8, B, NCHUNK, 64]
    Yc = Yc.reshape(NCORES, B, NCHUNK, 8, 8)             # [c, b, w, s, q]
    y = Yc.transpose(1, 4, 0, 2, 3).reshape(B, OUT_DIM)  # [b, q, c, w, s] -> v = q*256+c*32+8w+s
    return np.ascontiguousarray(y)
